# revision 28
# baseline (speedup 1.0000x reference)
"""Trainium2 Bass kernel: fused multi-head attention (dense transformer block).

Reference computation (per batch element b of 8, one NeuronCore each):
    qkv = x @ w_qkv.T                  # [1024, 2304]
    q, k, v = split(qkv); reshape to 12 heads x 64 dims
    s = q @ k.T (unscaled); p = softmax(s); o = p @ v
    out = concat_heads(o) @ w_fc.T + b_fc

Layout strategy (all per-core):
  - All operands arrive HOST-pre-transposed (xT, w_qkvT, w_fcT), so the kernel
    has zero PE transposes; the TensorEngine does only real matmuls.
  - Inputs are DMA'd DIRECTLY into f32r/bf16 SBUF tiles (f32r is bit-identical
    to f32; wfcT is converted to bf16 on host) -- no on-chip cast copies.
  - Input loads are BATCHED into few large multi-dim-AP descriptors (the Sync
    engine costs ~600ns per dma_start, so descriptor COUNT paces startup):
    xT in 2 (seq halves), wq per pair in 2, wvT in 1, wfcT in 1.
  - Transposed dataflow: qT/kT are [head_dim, seq]; scores S_T[k, q] (keys on
    partitions) so exp needs no transpose and P_T feeds P@V as moving operand.
  - S matmuls (contraction 64) are issued xi-alternated with tile_position
    (0,0)/(64,0) so the PE can overlap them through disjoint row-group halves.
  - Softmax skips max-subtraction (|scores| ~ 70 < 88 overflow limit); the
    denominator is free via a ones-column appended to V (P@V row 64 = sum_k P).
  - Normalize: stage po to SBUF (frees PSUM fast), denominator row through a
    DRAM-bounce reshape to [128,8] for a WIDE reciprocal (a [1,1024]
    single-partition DVE op costs ~6.5us!), DMA broadcast, one DVE multiply.
  - fc accumulates per-head-pair groups into an SBUF f32 accumulator (DVE
    adds, bias folded into the first pass), spread across later pairs'
    chunk streams so the serial fc tail is short.
  - Startup: full-array dummy matmuls keep the PE HAM clock at 8/8 through
    the DMA-bound window (woven between the pair-0 qk chains); a dummy exp
    preloads the ACT table set.  More dummies after the last drain keep the
    clock up through the ~10us normalize-chain flight so the final fc pass
    runs at full rate.
  - Precision: qkv + scores in float32r (TF32-like, full PE speed); P, V, ao,
    w_fc in bf16.  End-to-end ~3e-3 max rel err.
  - Measured: 241406 ns HW exec (prior-session baseline 263-268us), rel err
    3.13e-3; main loop runs within ~7% of the 85%-duty-throttled PE roofline.
"""

import numpy as np
import ml_dtypes
import concourse.bacc as bacc
import concourse.mybir as mybir
import concourse.tile as tile
from concourse.bass_utils import run_bass_kernel_spmd

SEQ = 1024
DIM = 768
H = 12
DH = 64
NT = SEQ // 128  # 8  seq chunks
DT = DIM // 128  # 6  dim chunks / head pairs
VA = H * (DH + 1)  # 780: v with ones column per head
NPAIR = 6
N_DUMMY = 12

f32 = mybir.dt.float32
f32r = mybir.dt.float32r
bf16 = mybir.dt.bfloat16
EXP = mybir.ActivationFunctionType.Exp


def build():
    nc = bacc.Bacc("TRN2", target_bir_lowering=False, debug=False)
    xT_d = nc.dram_tensor("xT", [DIM, SEQ], f32r, kind="ExternalInput")
    wqkT_d = nc.dram_tensor("wqkT", [DIM, 2 * DIM], f32r, kind="ExternalInput")
    wvT_d = nc.dram_tensor("wvT", [DIM, DIM], f32r, kind="ExternalInput")
    wfcT_d = nc.dram_tensor("wfcT", [DIM, DIM], bf16, kind="ExternalInput")
    bfc_d = nc.dram_tensor("b_fc", [1, DIM], f32, kind="ExternalInput")
    out_d = nc.dram_tensor("out", [SEQ, DIM], f32, kind="ExternalOutput")

    with tile.TileContext(nc) as tc:
        with (
            tc.tile_pool(name="const", bufs=1) as constp,
            tc.tile_pool(name="persist", bufs=1) as persist,
            tc.tile_pool(name="work", bufs=1) as work,
            tc.tile_pool(name="dsc", bufs=1, space="DRAM") as dscp,
            tc.tile_pool(name="ps", bufs=1, space="PSUM") as psp,
        ):
            # ---- warm-up: dummy exp preloads ACT tables; full-array dummy
            # matmuls keep the PE busy (HAM at 8/8) while DMAs stream in.
            # The startup is DMA-bandwidth-bound for ~15us, so dummies are
            # also WOVEN between the pair-0 qk chains below (only after a
            # chain closes -- a dummy inside an open chain would deadlock
            # on the rotating "mm" PSUM buffers).
            dmy = constp.tile([128, 512], f32, tag="dmy")
            nc.gpsimd.memset(dmy[:], 0.25)
            dmy_o = constp.tile([1, 512], bf16, tag="dmyo")
            nc.scalar.activation(dmy_o[:], dmy[0:1, :], EXP)
            dmy_r = constp.tile([128, 512], f32r, tag="dmyr")
            nc.vector.tensor_copy(dmy_r[:], dmy[:])

            def dummy_run(n):
                for _ in range(n):
                    psd = psp.tile([128, 512], f32, tag="mm", bufs=2,
                                   name="warm")
                    nc.tensor.matmul(psd[:], dmy_r[:, 0:128], dmy_r[:],
                                     start=True, stop=True)

            dummy_run(N_DUMMY)

            wq = {}  # p -> [q_tile, k_tile]: each [128, 6*128] f32r, j-major

            def load_wq_half(p, ci):
                t = work.tile([128, DT * 128], f32r, tag=("wqq", "wqk")[ci],
                              bufs=2, name=f"wq{'qk'[ci]}{p}")
                nc.sync.dma_start(
                    t[:].rearrange("r (j c) -> r j c", c=128),
                    wqkT_d.ap()[:, ci * DIM + p * 128:ci * DIM + (p + 1) * 128]
                    .rearrange("(j r) c -> r j c", r=128))
                wq.setdefault(p, [None, None])[ci] = t

            def load_wq(p):
                load_wq_half(p, 0)
                load_wq_half(p, 1)

            # xT loaded in j-triple quarters so the first qk accumulation
            # (js 0-2, h2=0) can start as early as possible.
            xT_all = persist.tile([128, DT * SEQ], f32r, tag="xT", name="xT")
            xT3 = xT_all[:].rearrange("r (j c) -> r j c", c=SEQ)

            def xs(j, lo, hi):
                return xT_all[:, j * SEQ + lo:j * SEQ + hi]

            def load_x_part(h2, jlo, jhi):
                nc.sync.dma_start(
                    xT3[:, jlo:jhi, h2 * 512:(h2 + 1) * 512],
                    xT_d.ap()[jlo * 128:jhi * 128, h2 * 512:(h2 + 1) * 512]
                    .rearrange("(j r) c -> r j c", r=128))

            load_wq_half(0, 0)
            load_x_part(0, 0, 3)
            load_x_part(0, 3, 6)
            load_wq_half(0, 1)
            load_x_part(1, 0, 3)
            load_x_part(1, 3, 6)
            load_wq(1)

            bias_row = constp.tile([1, DIM], f32, tag="brow")
            nc.sync.dma_start(bias_row[:], bfc_d.ap())

            wvT_all = persist.tile([128, DT * DIM], f32r, tag="wvT",
                                   name="wvT")
            nc.sync.dma_start(
                wvT_all[:].rearrange("r (j c) -> r j c", c=DIM),
                wvT_d.ap().rearrange("(j r) c -> r j c", r=128))

            def vs(j, lo, hi):
                return wvT_all[:, j * DIM + lo:j * DIM + hi]

            wfcT_all = persist.tile([128, DT * DIM], bf16, tag="wfcT",
                                    name="wfcT")
            nc.sync.dma_start(
                wfcT_all[:].rearrange("r (j c) -> r j c", c=DIM),
                wfcT_d.ap().rearrange("(j r) c -> r j c", r=128))

            def ws(j, lo, hi):
                return wfcT_all[:, j * DIM + lo:j * DIM + hi]

            bias_bc = constp.tile([128, DIM], f32, tag="bbc")
            nc.gpsimd.partition_broadcast(bias_bc[:], bias_row[:],
                                          channels=128)

            # ---- persistent tensors
            va = [persist.tile([128, VA], bf16, tag=f"va{nt}", name=f"va{nt}")
                  for nt in range(NT)]
            aoT = [persist.tile([128, SEQ], bf16, tag=f"ao{j}", name=f"aoT{j}")
                   for j in range(DT)]
            fc_acc = [persist.tile([128, DIM], f32, tag=f"acc{nt}",
                                   name=f"acc{nt}") for nt in range(NT)]

            # ---- qkv matmul parts -------------------------------------
            def qk_parts(p, tiles):
                """q_T/k_T matmuls for pair p, split into 3-matmul halves
                so the background stream stays fine-grained."""
                def qkmm(half, ci, h2, js, box):
                    def go():
                        if js[0] == 0:
                            box["ps"] = psp.tile([128, 512], f32, tag="mm",
                                                 bufs=2, name="ps_qk")
                        ps = box["ps"]
                        src = wq[p][ci]
                        for j in js:
                            nc.tensor.matmul(
                                ps[:],
                                src[:, j * 128:(j + 1) * 128],
                                xs(j, h2 * 512, (h2 + 1) * 512),
                                start=(j == 0), stop=(j == DT - 1))
                        if js[-1] != DT - 1:
                            return
                        if h2 == 0:
                            t = work.tile([128, SEQ], f32r,
                                          tag=f"qk_{half}{p % 2}", bufs=1,
                                          name=f"qk{half}{p}")
                            tiles[half] = t
                        nc.vector.tensor_copy(
                            tiles[half][:, h2 * 512:(h2 + 1) * 512], ps[:])
                    return go

                def prefetch():
                    if p + 2 < NPAIR:
                        load_wq(p + 2)

                parts = []
                for half, ci, h2 in (("q", 0, 0), ("k", 1, 0),
                                     ("q", 0, 1), ("k", 1, 1)):
                    box = {}
                    parts.append(qkmm(half, ci, h2, (0, 1, 2), box))
                    parts.append(qkmm(half, ci, h2, (3, 4, 5), box))
                parts.append(prefetch)
                return parts

            def v_parts(nts):
                """v natural [128n, 12h x 64d] + ones column -> va (bf16)."""
                def vp(nt, h2):
                    lo, hi = (0, 512) if h2 == 0 else (512, 768)

                    def go():
                        psv = psp.tile([128, hi - lo], f32, tag="mm", bufs=2,
                                       name="psv")
                        for j in range(DT):
                            nc.tensor.matmul(psv[:],
                                             xs(j, nt * 128, (nt + 1) * 128),
                                             vs(j, lo, hi),
                                             start=(j == 0),
                                             stop=(j == DT - 1))
                        va3 = va[nt][:].rearrange("p (h c) -> p h c", c=DH + 1)
                        nc.vector.tensor_copy(
                            va3[:, lo // DH:hi // DH, 0:DH],
                            psv[:].rearrange("p (h c) -> p h c", c=DH))
                        if h2 == 1:
                            nc.gpsimd.memset(va3[:, :, DH:DH + 1], 1.0)
                    return go
                return [vp(nt, h2) for nt in nts for h2 in range(2)]

            # ---- fc accumulation passes: `js` head-pairs' contribution to
            # every seq-chunk accumulator.  One [128,768] PSUM tile per nt
            # (bank-aligned halves for the matmuls) and a SINGLE wide DVE
            # add -- the adds, not the matmuls, pace the fc tail.
            def fc_parts(js, first, last):
                def fp(nt):
                    def go():
                        psy = psp.tile([128, DIM], f32, tag="mm", bufs=2,
                                       name="psy")
                        for lo, hi in ((0, 512), (512, DIM)):
                            for i, j in enumerate(js):
                                nc.tensor.matmul(
                                    psy[:, lo:hi],
                                    aoT[j][:, nt * 128:(nt + 1) * 128],
                                    ws(j, lo, hi),
                                    start=(i == 0), stop=(i == len(js) - 1))
                        acc = fc_acc[nt]
                        if first:
                            nc.vector.tensor_add(acc[:], psy[:], bias_bc[:])
                        else:
                            nc.vector.tensor_add(acc[:], psy[:], acc[:])
                        if last:
                            nc.sync.dma_start(
                                out_d.ap()[nt * 128:(nt + 1) * 128, :],
                                acc[:])
                    return go
                return [fp(nt) for nt in range(NT)]

            # ---- softmax normalize: stage PSUM->SBUF, then a wide
            # reciprocal (a [1,1024] single-partition DVE op costs ~6.5us;
            # the [128,8] reshape keeps the reciprocal at ~0.2us).  The
            # reshape is ONE SBUF->SBUF partition-scatter DMA; the
            # partition-broadcast still needs a DRAM bounce.  For the tail
            # drains (pair 5) the staging copy runs on the Scalar engine,
            # which has finished all exps by then.
            def drain_po(p, xi, po, use_scalar=False):
                if use_scalar:
                    # Tail variant (scalar/DVE/gpsimd all idle): normalize
                    # with ZERO DMA hops.  The denominator row is copied to
                    # partition 0 of its own tile (partition_broadcast
                    # broadcasts partition 0), broadcast across 64
                    # partitions on the gpsimd, inverted on the DVE, and
                    # multiplied in.  ~5us vs ~11.7us for the DMA-bounce
                    # chain this replaces.
                    den_row = work.tile([1, SEQ], f32, tag="denrow", bufs=1,
                                        name="denrow")
                    nc.scalar.copy(den_row[:], po[DH:DH + 1, :])
                    st = work.tile([DH + 1, SEQ], f32, tag="stage", bufs=2,
                                   name="st2")
                    nc.scalar.copy(st[0:DH, :], po[0:DH, :])
                    den_bc = work.tile([64, SEQ], f32, tag="bc", bufs=2,
                                       name="denbc")
                    nc.gpsimd.partition_broadcast(den_bc[:], den_row[:],
                                                  channels=64)
                    recip_bc = work.tile([64, SEQ], f32, tag="bc",
                                         bufs=2, name="recipbc")
                    nc.vector.reciprocal(recip_bc[:], den_bc[:])
                    nc.vector.tensor_mul(
                        aoT[p][xi * 64:(xi + 1) * 64, :], st[0:DH, :],
                        recip_bc[:])
                    return
                st = work.tile([DH + 1, SEQ], f32, tag="stage", bufs=2,
                               name="st")
                nc.vector.tensor_copy(st[:], po[:])
                den8 = work.tile([128, 8], f32, tag="den8", bufs=2,
                                 name="den8")
                nc.sync.dma_start(den8[:], st[DH:DH + 1, :])
                recip8 = work.tile([128, 8], f32, tag="recip8", bufs=2,
                                   name="recip8")
                nc.vector.reciprocal(recip8[:], den8[:])
                dsc2 = dscp.tile([1, SEQ], f32, tag="dsc2", bufs=2,
                                 name="dsc2")
                nc.sync.dma_start(
                    dsc2[:].rearrange("a (p c) -> (a p) c", c=8), recip8[:])
                bc_sb = work.tile([64, SEQ], f32, tag="bc", bufs=2,
                                  name="bc_sb")
                nc.sync.dma_start(bc_sb[:], dsc2[:].broadcast_to([64, SEQ]))
                nc.vector.tensor_mul(
                    aoT[p][xi * 64:(xi + 1) * 64, :], st[0:DH, :], bc_sb[:])

            # ---- pipelined pair loop -----------------------------------
            def pair_step(p, qk, PT_prev, bg=()):
                """bg parts + PV(p-1) + S(p) quad + exp per chunk."""
                qt, kt = qk["q"], qk["k"]
                L = len(bg)
                PT = {}
                po = {}
                if PT_prev is not None:
                    for xi in range(2):
                        po[xi] = psp.tile([DH + 1, SEQ], f32, tag=f"o{xi}",
                                          bufs=1, name=f"po{xi}")
                def do_pv(c):
                    for xi in range(2):
                        hX = 2 * (p - 1) + xi
                        va_h = va[c][:, hX * (DH + 1):(hX + 1) * (DH + 1)]
                        for h2 in range(2):
                            nc.tensor.matmul(
                                po[xi][:, h2 * 512:(h2 + 1) * 512],
                                va_h,
                                PT_prev[(xi, c)][:, h2 * 512:
                                                 (h2 + 1) * 512],
                                start=(c == 0), stop=(c == NT - 1))

                def do_s(c):
                    ps = {}
                    for xi in range(2):
                        ps[xi] = psp.tile([128, SEQ], f32, tag="mm", bufs=2,
                                          name=f"ps_s{xi}")
                    for h2 in range(2):
                        for xi in range(2):
                            ro = xi * 64
                            nc.tensor.matmul(
                                ps[xi][:, h2 * 512:(h2 + 1) * 512],
                                kt[ro:ro + 64, c * 128:(c + 1) * 128],
                                qt[ro:ro + 64, h2 * 512:(h2 + 1) * 512],
                                start=True, stop=True,
                                tile_position=(ro, 0))
                    for xi in range(2):
                        pt = work.tile([128, SEQ], bf16, tag=f"pt{xi}_{c}",
                                       bufs=1, name="pt")
                        nc.scalar.activation(pt[:], ps[xi][:], EXP)
                        PT[(xi, c)] = pt

                for c in range(NT):
                    for i in range(L * c // NT, L * (c + 1) // NT):
                        bg[i]()
                    if PT_prev is not None:
                        do_pv(c)
                    do_s(c)
                if PT_prev is not None:
                    for xi in range(2):
                        drain_po(p - 1, xi, po[xi])
                return PT

            def pv_only(p, PT_prev, bg=()):
                """Last pair's PV, xi-major (no later exp depends on the pt
                tags).  The bg parts (fc pass 4) run in the xi=1 PV slots
                covering xi=0's drain chain; dummies after the xi=1 drain
                keep the HAM clock at 8/8 through the chain's ~10us flight
                so the final fc matmuls run at full rate."""
                L = len(bg)
                for xi in range(2):
                    po = psp.tile([DH + 1, SEQ], f32, tag="o0", bufs=1,
                                  name=f"po{xi}")
                    hX = 2 * p + xi
                    for t in range(4):
                        for c in (2 * t, 2 * t + 1):
                            va_h = va[c][:, hX * (DH + 1):
                                         (hX + 1) * (DH + 1)]
                            for h2 in range(2):
                                nc.tensor.matmul(
                                    po[:, h2 * 512:(h2 + 1) * 512],
                                    va_h,
                                    PT_prev[(xi, c)][:, h2 * 512:
                                                     (h2 + 1) * 512],
                                    start=(c == 0), stop=(c == NT - 1))
                        if xi == 1:
                            for i in range(L * t // 4, L * (t + 1) // 4):
                                bg[i]()
                    drain_po(p, xi, po, use_scalar=True)
                    if xi == 1:
                        dummy_run(20)
                for f in fc_parts((p,), first=False, last=True):
                    f()

            def merge(a, b):
                out, ia, ib = [], 0, 0
                while ia < len(a) or ib < len(b):
                    if ia * len(b) <= ib * len(a) and ia < len(a):
                        out.append(a[ia]); ia += 1
                    elif ib < len(b):
                        out.append(b[ib]); ib += 1
                    else:
                        out.append(a[ia]); ia += 1
                return out

            qk_tiles = {p: {} for p in range(NPAIR)}
            qk0 = qk_parts(0, qk_tiles[0])
            for i, f in enumerate(qk0):
                f()
                if i in (1, 3, 5):  # after each closed qk chain
                    dummy_run(10)

            # fc accumulation spread across the pair loop: pass {0,1} as
            # soon as aoT[1] exists, {2,3} in pair 5, {4} during PV(5),
            # {5} at the very end.  v-parts are concatenated AFTER qk parts
            # so early background work never waits on DMAs that arrive late
            # in the startup stream.
            bg_sched = {
                0: qk_parts(1, qk_tiles[1]) + v_parts(range(0, 6)),
                1: v_parts(range(6, 8)) + qk_parts(2, qk_tiles[2]),
                2: qk_parts(3, qk_tiles[3]),
                3: merge(qk_parts(4, qk_tiles[4]),
                         fc_parts((0, 1), first=True, last=False)),
                4: qk_parts(5, qk_tiles[5]),
                5: fc_parts((2, 3), first=False, last=False),
            }
            PT_cur = None
            for p in range(NPAIR):
                PT_cur = pair_step(p, qk_tiles[p], PT_cur, bg_sched[p])
            pv_only(5, PT_cur, bg=fc_parts((4,), first=False, last=False))

    nc.compile()
    return nc


_NC = None
LAST_RESULTS = None  # BassKernelResults of the most recent run (for profiling)


def kernel(**inputs) -> np.ndarray:
    global _NC, LAST_RESULTS
    x = np.asarray(inputs["x"], dtype=np.float32)
    w_qkv = np.asarray(inputs["w_qkv"], dtype=np.float32)
    w_fc = np.asarray(inputs["w_fc"], dtype=np.float32)
    b_fc = np.ascontiguousarray(
        np.asarray(inputs["b_fc"], dtype=np.float32).reshape(1, DIM))

    wqkT = np.ascontiguousarray(w_qkv[0:2 * DIM].T)    # [768, 1536]
    wvT = np.ascontiguousarray(w_qkv[2 * DIM:].T)      # [768, 768]
    wfcT = np.ascontiguousarray(w_fc.T).astype(ml_dtypes.bfloat16)

    if _NC is None:
        _NC = build()
    nc = _NC

    in_maps = [
        {"xT": np.ascontiguousarray(x[b].T), "wqkT": wqkT, "wvT": wvT,
         "wfcT": wfcT, "b_fc": b_fc}
        for b in range(8)
    ]
    res = run_bass_kernel_spmd(nc, in_maps, core_ids=list(range(8)))
    LAST_RESULTS = res
    out = np.stack([r["out"] for r in res.results], axis=0)
    return out.astype(np.float32)


if __name__ == "__main__":
    rng = np.random.default_rng(0)
    ins = {
        "x": rng.standard_normal((8, SEQ, DIM), dtype=np.float32),
        "w_qkv": (rng.standard_normal((3 * DIM, DIM), dtype=np.float32)
                  * DIM ** -0.5),
        "w_fc": (rng.standard_normal((DIM, DIM), dtype=np.float32)
                 * DIM ** -0.5),
        "b_fc": (rng.standard_normal((DIM,), dtype=np.float32) * 0.02),
    }
    out = kernel(**ins)
    print("out", out.shape, out.dtype)


# revision 31
# speedup vs baseline: 1.0210x; 1.0210x over previous
"""Trainium2 Bass kernel: fused multi-head attention (dense transformer block).

Reference computation (per batch element b of 8, one NeuronCore each):
    qkv = x @ w_qkv.T                  # [1024, 2304]
    q, k, v = split(qkv); reshape to 12 heads x 64 dims
    s = q @ k.T (unscaled); p = softmax(s); o = p @ v
    out = concat_heads(o) @ w_fc.T + b_fc

Layout strategy (all per-core):
  - All operands arrive HOST-pre-transposed (xT, w_qkvT, w_fcT), so the kernel
    has zero PE transposes; the TensorEngine does only real matmuls.
  - Inputs are DMA'd DIRECTLY into f32r/bf16 SBUF tiles (f32r is bit-identical
    to f32; wfcT is converted to bf16 on host) -- no on-chip cast copies.
  - Input loads are BATCHED into few large multi-dim-AP descriptors (the Sync
    engine costs ~600ns per dma_start, so descriptor COUNT paces startup):
    xT in 2 (seq halves), wq per pair in 2, wvT in 1, wfcT in 1.
  - Transposed dataflow: qT/kT are [head_dim, seq]; scores S_T[k, q] (keys on
    partitions) so exp needs no transpose and P_T feeds P@V as moving operand.
  - S matmuls (contraction 64) are issued xi-alternated with tile_position
    (0,0)/(64,0) so the PE can overlap them through disjoint row-group halves.
  - Softmax skips max-subtraction (|scores| ~ 70 < 88 overflow limit); the
    denominator is free via a ones-column appended to V (P@V row 64 = sum_k P).
  - Normalize: stage po to SBUF (frees PSUM fast), denominator row through a
    DRAM-bounce reshape to [128,8] for a WIDE reciprocal (a [1,1024]
    single-partition DVE op costs ~6.5us!), DMA broadcast, one DVE multiply.
  - fc accumulates per-head-pair groups into an SBUF f32 accumulator (DVE
    adds, bias folded into the first pass), spread across later pairs'
    chunk streams so the serial fc tail is short.
  - Startup: full-array dummy matmuls keep the PE HAM clock at 8/8 through
    the DMA-bound window (woven between the pair-0 qk chains); a dummy exp
    preloads the ACT table set.  More dummies after the last drain keep the
    clock up through the ~10us normalize-chain flight so the final fc pass
    runs at full rate.
  - Precision: qkv + scores in float32r (TF32-like, full PE speed); P, V, ao,
    w_fc in bf16.  End-to-end ~3e-3 max rel err.
  - Measured: 241406 ns HW exec (prior-session baseline 263-268us), rel err
    3.13e-3; main loop runs within ~7% of the 85%-duty-throttled PE roofline.
    (DVE RECIPROCAL costs ~6.4ns/free-element regardless of partitions --
    the [128,8] reshape for the softmax denominators is load-bearing.)
"""

import numpy as np
import ml_dtypes
import concourse.bacc as bacc
import concourse.mybir as mybir
import concourse.tile as tile
from concourse.bass_utils import run_bass_kernel_spmd

SEQ = 1024
DIM = 768
H = 12
DH = 64
NT = SEQ // 128  # 8  seq chunks
DT = DIM // 128  # 6  dim chunks / head pairs
VA = H * (DH + 1)  # 780: v with ones column per head
NPAIR = 6
N_DUMMY = 12

f32 = mybir.dt.float32
f32r = mybir.dt.float32r
bf16 = mybir.dt.bfloat16
EXP = mybir.ActivationFunctionType.Exp


def build():
    nc = bacc.Bacc("TRN2", target_bir_lowering=False, debug=False)
    xT_d = nc.dram_tensor("xT", [DIM, SEQ], f32r, kind="ExternalInput")
    wqkT_d = nc.dram_tensor("wqkT", [DIM, 2 * DIM], f32r, kind="ExternalInput")
    wvT_d = nc.dram_tensor("wvT", [DIM, DIM], f32r, kind="ExternalInput")
    wfcT_d = nc.dram_tensor("wfcT", [DIM, DIM], bf16, kind="ExternalInput")
    bfc_d = nc.dram_tensor("b_fc", [1, DIM], f32, kind="ExternalInput")
    out_d = nc.dram_tensor("out", [SEQ, DIM], f32, kind="ExternalOutput")

    with tile.TileContext(nc) as tc:
        with (
            tc.tile_pool(name="const", bufs=1) as constp,
            tc.tile_pool(name="persist", bufs=1) as persist,
            tc.tile_pool(name="work", bufs=1) as work,
            tc.tile_pool(name="dsc", bufs=1, space="DRAM") as dscp,
            tc.tile_pool(name="ps", bufs=1, space="PSUM") as psp,
        ):
            # ---- warm-up: dummy exp preloads ACT tables; full-array dummy
            # matmuls keep the PE busy (HAM at 8/8) while DMAs stream in.
            # The startup is DMA-bandwidth-bound for ~15us, so dummies are
            # also WOVEN between the pair-0 qk chains below (only after a
            # chain closes -- a dummy inside an open chain would deadlock
            # on the rotating "mm" PSUM buffers).
            dmy = constp.tile([128, 512], f32, tag="dmy")
            nc.gpsimd.memset(dmy[:], 0.25)
            dmy_o = constp.tile([1, 512], bf16, tag="dmyo")
            nc.scalar.activation(dmy_o[:], dmy[0:1, :], EXP)
            dmy_r = constp.tile([128, 512], f32r, tag="dmyr")
            nc.vector.tensor_copy(dmy_r[:], dmy[:])

            def dummy_run(n):
                for _ in range(n):
                    psd = psp.tile([128, 512], f32, tag="mm", bufs=2,
                                   name="warm")
                    nc.tensor.matmul(psd[:], dmy_r[:, 0:128], dmy_r[:],
                                     start=True, stop=True)

            dummy_run(N_DUMMY)

            wq = {}  # p -> [q_tile, k_tile]: each [128, 6*128] f32r, j-major

            def load_wq_half(p, ci):
                t = work.tile([128, DT * 128], f32r, tag=("wqq", "wqk")[ci],
                              bufs=2, name=f"wq{'qk'[ci]}{p}")
                nc.sync.dma_start(
                    t[:].rearrange("r (j c) -> r j c", c=128),
                    wqkT_d.ap()[:, ci * DIM + p * 128:ci * DIM + (p + 1) * 128]
                    .rearrange("(j r) c -> r j c", r=128))
                wq.setdefault(p, [None, None])[ci] = t

            def load_wq(p):
                load_wq_half(p, 0)
                load_wq_half(p, 1)

            # xT loaded in j-triple quarters so the first qk accumulation
            # (js 0-2, h2=0) can start as early as possible.
            xT_all = persist.tile([128, DT * SEQ], f32r, tag="xT", name="xT")
            xT3 = xT_all[:].rearrange("r (j c) -> r j c", c=SEQ)

            def xs(j, lo, hi):
                return xT_all[:, j * SEQ + lo:j * SEQ + hi]

            def load_x_part(h2, jlo, jhi):
                nc.sync.dma_start(
                    xT3[:, jlo:jhi, h2 * 512:(h2 + 1) * 512],
                    xT_d.ap()[jlo * 128:jhi * 128, h2 * 512:(h2 + 1) * 512]
                    .rearrange("(j r) c -> r j c", r=128))

            load_wq_half(0, 0)
            load_x_part(0, 0, 3)
            load_x_part(0, 3, 6)
            load_wq_half(0, 1)
            load_x_part(1, 0, 3)
            load_x_part(1, 3, 6)
            load_wq(1)

            bias_row = constp.tile([1, DIM], f32, tag="brow")
            nc.sync.dma_start(bias_row[:], bfc_d.ap())

            wvT_all = persist.tile([128, DT * DIM], f32r, tag="wvT",
                                   name="wvT")
            nc.sync.dma_start(
                wvT_all[:].rearrange("r (j c) -> r j c", c=DIM),
                wvT_d.ap().rearrange("(j r) c -> r j c", r=128))

            def vs(j, lo, hi):
                return wvT_all[:, j * DIM + lo:j * DIM + hi]

            wfcT_all = persist.tile([128, DT * DIM], bf16, tag="wfcT",
                                    name="wfcT")
            nc.sync.dma_start(
                wfcT_all[:].rearrange("r (j c) -> r j c", c=DIM),
                wfcT_d.ap().rearrange("(j r) c -> r j c", r=128))

            def ws(j, lo, hi):
                return wfcT_all[:, j * DIM + lo:j * DIM + hi]

            bias_bc = constp.tile([128, DIM], f32, tag="bbc")
            nc.gpsimd.partition_broadcast(bias_bc[:], bias_row[:],
                                          channels=128)

            # ---- persistent tensors
            va = [persist.tile([128, VA], bf16, tag=f"va{nt}", name=f"va{nt}")
                  for nt in range(NT)]
            aoT = [persist.tile([128, SEQ], bf16, tag=f"ao{j}", name=f"aoT{j}")
                   for j in range(DT)]
            fc_acc = [persist.tile([128, DIM], f32, tag=f"acc{nt}",
                                   name=f"acc{nt}") for nt in range(NT)]

            # ---- qkv matmul parts -------------------------------------
            def qk_parts(p, tiles):
                """q_T/k_T matmuls for pair p, split into 3-matmul halves
                so the background stream stays fine-grained."""
                def qkmm(half, ci, h2, js, box):
                    def go():
                        if js[0] == 0:
                            box["ps"] = psp.tile([128, 512], f32, tag="mm",
                                                 bufs=2, name="ps_qk")
                        ps = box["ps"]
                        src = wq[p][ci]
                        for j in js:
                            nc.tensor.matmul(
                                ps[:],
                                src[:, j * 128:(j + 1) * 128],
                                xs(j, h2 * 512, (h2 + 1) * 512),
                                start=(j == 0), stop=(j == DT - 1))
                        if js[-1] != DT - 1:
                            return
                        if h2 == 0:
                            t = work.tile([128, SEQ], f32r,
                                          tag=f"qk_{half}{p % 2}", bufs=1,
                                          name=f"qk{half}{p}")
                            tiles[half] = t
                        nc.vector.tensor_copy(
                            tiles[half][:, h2 * 512:(h2 + 1) * 512], ps[:])
                    return go

                def prefetch():
                    if p + 2 < NPAIR:
                        load_wq(p + 2)

                parts = []
                for half, ci, h2 in (("q", 0, 0), ("k", 1, 0),
                                     ("q", 0, 1), ("k", 1, 1)):
                    box = {}
                    parts.append(qkmm(half, ci, h2, (0, 1, 2), box))
                    parts.append(qkmm(half, ci, h2, (3, 4, 5), box))
                parts.append(prefetch)
                return parts

            def v_parts(nts):
                """v natural [128n, 12h x 64d] + ones column -> va (bf16)."""
                def vp(nt, h2):
                    lo, hi = (0, 512) if h2 == 0 else (512, 768)

                    def go():
                        psv = psp.tile([128, hi - lo], f32, tag="mm", bufs=2,
                                       name="psv")
                        for j in range(DT):
                            nc.tensor.matmul(psv[:],
                                             xs(j, nt * 128, (nt + 1) * 128),
                                             vs(j, lo, hi),
                                             start=(j == 0),
                                             stop=(j == DT - 1))
                        va3 = va[nt][:].rearrange("p (h c) -> p h c", c=DH + 1)
                        nc.vector.tensor_copy(
                            va3[:, lo // DH:hi // DH, 0:DH],
                            psv[:].rearrange("p (h c) -> p h c", c=DH))
                        if h2 == 1:
                            nc.gpsimd.memset(va3[:, :, DH:DH + 1], 1.0)
                    return go
                return [vp(nt, h2) for nt in nts for h2 in range(2)]

            # ---- fc accumulation passes: `js` head-pairs' contribution to
            # every seq-chunk accumulator.  One [128,768] PSUM tile per nt
            # (bank-aligned halves for the matmuls) and a SINGLE wide DVE
            # add -- the adds, not the matmuls, pace the fc tail.
            def fc_parts(js, first, last):
                def fp(nt):
                    def go():
                        psy = psp.tile([128, DIM], f32, tag="mm", bufs=2,
                                       name="psy")
                        for lo, hi in ((0, 512), (512, DIM)):
                            for i, j in enumerate(js):
                                nc.tensor.matmul(
                                    psy[:, lo:hi],
                                    aoT[j][:, nt * 128:(nt + 1) * 128],
                                    ws(j, lo, hi),
                                    start=(i == 0), stop=(i == len(js) - 1))
                        acc = fc_acc[nt]
                        if first:
                            nc.vector.tensor_add(acc[:], psy[:], bias_bc[:])
                        else:
                            nc.vector.tensor_add(acc[:], psy[:], acc[:])
                        if last:
                            if nt == NT - 1:
                                # final chunk: two half-width descriptors so
                                # the last transfer (which gates the end
                                # barrier) halves
                                nc.sync.dma_start(
                                    out_d.ap()[nt * 128:(nt + 1) * 128,
                                               0:512], acc[:, 0:512])
                                nc.sync.dma_start(
                                    out_d.ap()[nt * 128:(nt + 1) * 128,
                                               512:DIM], acc[:, 512:DIM])
                            else:
                                nc.sync.dma_start(
                                    out_d.ap()[nt * 128:(nt + 1) * 128, :],
                                    acc[:])
                    return go
                return [fp(nt) for nt in range(NT)]

            # ---- softmax normalize: stage PSUM->SBUF, then a wide
            # reciprocal (a [1,1024] single-partition DVE op costs ~6.5us;
            # the [128,8] reshape keeps the reciprocal at ~0.2us).  The
            # reshape is ONE SBUF->SBUF partition-scatter DMA; the
            # partition-broadcast still needs a DRAM bounce.  For the tail
            # drains (pair 5) the staging copy runs on the Scalar engine,
            # which has finished all exps by then.
            def drain_po(p, xi, po, use_scalar=False):
                st = work.tile([DH + 1, SEQ], f32, tag="stage", bufs=2,
                               name="st")
                if use_scalar:
                    nc.scalar.copy(st[:], po[:])
                else:
                    nc.vector.tensor_copy(st[:], po[:])
                den8 = work.tile([128, 8], f32, tag="den8", bufs=2,
                                 name="den8")
                nc.sync.dma_start(den8[:], st[DH:DH + 1, :])
                recip8 = work.tile([128, 8], f32, tag="recip8", bufs=2,
                                   name="recip8")
                nc.vector.reciprocal(recip8[:], den8[:])
                dsc2 = dscp.tile([1, SEQ], f32, tag="dsc2", bufs=2,
                                 name="dsc2")
                nc.sync.dma_start(
                    dsc2[:].rearrange("a (p c) -> (a p) c", c=8), recip8[:])
                bc_sb = work.tile([64, SEQ], f32, tag="bc", bufs=2,
                                  name="bc_sb")
                if use_scalar:
                    # tail drains: the broadcast read is the chain's longest
                    # hop (~3.3us); two descriptors spread its packets over
                    # twice the DMA queues
                    nc.sync.dma_start(bc_sb[0:32, :],
                                      dsc2[:].broadcast_to([32, SEQ]))
                    nc.sync.dma_start(bc_sb[32:64, :],
                                      dsc2[:].broadcast_to([32, SEQ]))
                else:
                    nc.sync.dma_start(bc_sb[:],
                                      dsc2[:].broadcast_to([64, SEQ]))
                nc.vector.tensor_mul(
                    aoT[p][xi * 64:(xi + 1) * 64, :], st[0:DH, :], bc_sb[:])

            # ---- pipelined pair loop -----------------------------------
            def pair_step(p, qk, PT_prev, bg=()):
                """bg parts + PV(p-1) + S(p) quad + exp per chunk."""
                qt, kt = qk["q"], qk["k"]
                L = len(bg)
                PT = {}
                po = {}
                if PT_prev is not None:
                    for xi in range(2):
                        po[xi] = psp.tile([DH + 1, SEQ], f32, tag=f"o{xi}",
                                          bufs=1, name=f"po{xi}")
                def do_pv(c):
                    for xi in range(2):
                        hX = 2 * (p - 1) + xi
                        va_h = va[c][:, hX * (DH + 1):(hX + 1) * (DH + 1)]
                        for h2 in range(2):
                            nc.tensor.matmul(
                                po[xi][:, h2 * 512:(h2 + 1) * 512],
                                va_h,
                                PT_prev[(xi, c)][:, h2 * 512:
                                                 (h2 + 1) * 512],
                                start=(c == 0), stop=(c == NT - 1))

                def do_s(c):
                    ps = {}
                    for xi in range(2):
                        ps[xi] = psp.tile([128, SEQ], f32, tag="mm", bufs=2,
                                          name=f"ps_s{xi}")
                    for h2 in range(2):
                        for xi in range(2):
                            ro = xi * 64
                            nc.tensor.matmul(
                                ps[xi][:, h2 * 512:(h2 + 1) * 512],
                                kt[ro:ro + 64, c * 128:(c + 1) * 128],
                                qt[ro:ro + 64, h2 * 512:(h2 + 1) * 512],
                                start=True, stop=True,
                                tile_position=(ro, 0))
                    for xi in range(2):
                        pt = work.tile([128, SEQ], bf16, tag=f"pt{xi}_{c}",
                                       bufs=1, name="pt")
                        nc.scalar.activation(pt[:], ps[xi][:], EXP)
                        PT[(xi, c)] = pt

                for c in range(NT):
                    for i in range(L * c // NT, L * (c + 1) // NT):
                        bg[i]()
                    if PT_prev is not None:
                        do_pv(c)
                    do_s(c)
                if PT_prev is not None:
                    for xi in range(2):
                        drain_po(p - 1, xi, po[xi])
                return PT

            def pv_only(p, PT_prev, bg=()):
                """Last pair's PV, xi-major (no later exp depends on the pt
                tags).  The bg parts (fc pass 4) run in the xi=1 PV slots
                covering xi=0's drain chain; dummies after the xi=1 drain
                keep the HAM clock at 8/8 through the chain's ~10us flight
                so the final fc matmuls run at full rate."""
                L = len(bg)
                for xi in range(2):
                    po = psp.tile([DH + 1, SEQ], f32, tag="o0", bufs=1,
                                  name=f"po{xi}")
                    hX = 2 * p + xi
                    for t in range(4):
                        for c in (2 * t, 2 * t + 1):
                            va_h = va[c][:, hX * (DH + 1):
                                         (hX + 1) * (DH + 1)]
                            for h2 in range(2):
                                nc.tensor.matmul(
                                    po[:, h2 * 512:(h2 + 1) * 512],
                                    va_h,
                                    PT_prev[(xi, c)][:, h2 * 512:
                                                     (h2 + 1) * 512],
                                    start=(c == 0), stop=(c == NT - 1))
                        if xi == 1:
                            for i in range(L * t // 4, L * (t + 1) // 4):
                                bg[i]()
                    drain_po(p, xi, po, use_scalar=True)
                    if xi == 1:
                        dummy_run(20)
                for f in fc_parts((p,), first=False, last=True):
                    f()

            def merge(a, b):
                out, ia, ib = [], 0, 0
                while ia < len(a) or ib < len(b):
                    if ia * len(b) <= ib * len(a) and ia < len(a):
                        out.append(a[ia]); ia += 1
                    elif ib < len(b):
                        out.append(b[ib]); ib += 1
                    else:
                        out.append(a[ia]); ia += 1
                return out

            qk_tiles = {p: {} for p in range(NPAIR)}
            qk0 = qk_parts(0, qk_tiles[0])
            for i, f in enumerate(qk0):
                f()
                if i in (1, 3, 5):  # after each closed qk chain
                    dummy_run(10)

            # fc accumulation spread across the pair loop: pass {0,1} as
            # soon as aoT[1] exists, {2,3} in pair 5, {4} during PV(5),
            # {5} at the very end.  v-parts are concatenated AFTER qk parts
            # so early background work never waits on DMAs that arrive late
            # in the startup stream.
            bg_sched = {
                0: qk_parts(1, qk_tiles[1]) + v_parts(range(0, 6)),
                1: v_parts(range(6, 8)) + qk_parts(2, qk_tiles[2]),
                2: qk_parts(3, qk_tiles[3]),
                3: merge(qk_parts(4, qk_tiles[4]),
                         fc_parts((0, 1), first=True, last=False)),
                4: qk_parts(5, qk_tiles[5]),
                5: fc_parts((2, 3), first=False, last=False),
            }
            PT_cur = None
            for p in range(NPAIR):
                PT_cur = pair_step(p, qk_tiles[p], PT_cur, bg_sched[p])
            pv_only(5, PT_cur, bg=fc_parts((4,), first=False, last=False))

    nc.compile()
    return nc


_NC = None
LAST_RESULTS = None  # BassKernelResults of the most recent run (for profiling)


def kernel(**inputs) -> np.ndarray:
    global _NC, LAST_RESULTS
    x = np.asarray(inputs["x"], dtype=np.float32)
    w_qkv = np.asarray(inputs["w_qkv"], dtype=np.float32)
    w_fc = np.asarray(inputs["w_fc"], dtype=np.float32)
    b_fc = np.ascontiguousarray(
        np.asarray(inputs["b_fc"], dtype=np.float32).reshape(1, DIM))

    wqkT = np.ascontiguousarray(w_qkv[0:2 * DIM].T)    # [768, 1536]
    wvT = np.ascontiguousarray(w_qkv[2 * DIM:].T)      # [768, 768]
    wfcT = np.ascontiguousarray(w_fc.T).astype(ml_dtypes.bfloat16)

    if _NC is None:
        _NC = build()
    nc = _NC

    in_maps = [
        {"xT": np.ascontiguousarray(x[b].T), "wqkT": wqkT, "wvT": wvT,
         "wfcT": wfcT, "b_fc": b_fc}
        for b in range(8)
    ]
    res = run_bass_kernel_spmd(nc, in_maps, core_ids=list(range(8)))
    LAST_RESULTS = res
    out = np.stack([r["out"] for r in res.results], axis=0)
    return out.astype(np.float32)


if __name__ == "__main__":
    rng = np.random.default_rng(0)
    ins = {
        "x": rng.standard_normal((8, SEQ, DIM), dtype=np.float32),
        "w_qkv": (rng.standard_normal((3 * DIM, DIM), dtype=np.float32)
                  * DIM ** -0.5),
        "w_fc": (rng.standard_normal((DIM, DIM), dtype=np.float32)
                 * DIM ** -0.5),
        "b_fc": (rng.standard_normal((DIM,), dtype=np.float32) * 0.02),
    }
    out = kernel(**ins)
    print("out", out.shape, out.dtype)


# revision 32
# speedup vs baseline: 1.0245x; 1.0035x over previous
"""Trainium2 Bass kernel: fused multi-head attention (dense transformer block).

Reference computation (per batch element b of 8, one NeuronCore each):
    qkv = x @ w_qkv.T                  # [1024, 2304]
    q, k, v = split(qkv); reshape to 12 heads x 64 dims
    s = q @ k.T (unscaled); p = softmax(s); o = p @ v
    out = concat_heads(o) @ w_fc.T + b_fc

Layout strategy (all per-core):
  - All operands arrive HOST-pre-transposed (xT, w_qkvT, w_fcT), so the kernel
    has zero PE transposes; the TensorEngine does only real matmuls.
  - Inputs are DMA'd DIRECTLY into f32r/bf16 SBUF tiles (f32r is bit-identical
    to f32; wfcT is converted to bf16 on host) -- no on-chip cast copies.
  - Input loads are BATCHED into few large multi-dim-AP descriptors (the Sync
    engine costs ~600ns per dma_start, so descriptor COUNT paces startup):
    xT in 2 (seq halves), wq per pair in 2, wvT in 1, wfcT in 1.
  - Transposed dataflow: qT/kT are [head_dim, seq]; scores S_T[k, q] (keys on
    partitions) so exp needs no transpose and P_T feeds P@V as moving operand.
  - S matmuls (contraction 64) are issued xi-alternated with tile_position
    (0,0)/(64,0) so the PE can overlap them through disjoint row-group halves.
  - Softmax skips max-subtraction (|scores| ~ 70 < 88 overflow limit); the
    denominator is free via a ones-column appended to V (P@V row 64 = sum_k P).
  - Normalize: stage po to SBUF (frees PSUM fast), denominator row through a
    DRAM-bounce reshape to [128,8] for a WIDE reciprocal (a [1,1024]
    single-partition DVE op costs ~6.5us!), DMA broadcast, one DVE multiply.
  - fc accumulates per-head-pair groups into an SBUF f32 accumulator (DVE
    adds, bias folded into the first pass), spread across later pairs'
    chunk streams so the serial fc tail is short.
  - Startup: full-array dummy matmuls keep the PE HAM clock at 8/8 through
    the DMA-bound window (woven between the pair-0 qk chains); a dummy exp
    preloads the ACT table set.  More dummies after the last drain keep the
    clock up through the ~10us normalize-chain flight so the final fc pass
    runs at full rate.
  - Precision: qkv + scores in float32r (TF32-like, full PE speed); P, V, ao,
    w_fc in bf16.  End-to-end ~3e-3 max rel err.
  - Measured: 241406 ns HW exec (prior-session baseline 263-268us), rel err
    3.13e-3; main loop runs within ~7% of the 85%-duty-throttled PE roofline.
    Run-to-run variance is ~+/-2us (HAM throttle phases move between runs).
    Measured dead ends: DVE RECIPROCAL costs ~6.4ns/free-element regardless
    of partition count (the [128,8] denominator reshape is load-bearing);
    mixed-dtype DVE tensor ops hit a slow path; splitting the tail
    broadcast/out DMAs into more descriptors loses to descriptor+sem cost.
"""

import numpy as np
import ml_dtypes
import concourse.bacc as bacc
import concourse.mybir as mybir
import concourse.tile as tile
from concourse.bass_utils import run_bass_kernel_spmd

SEQ = 1024
DIM = 768
H = 12
DH = 64
NT = SEQ // 128  # 8  seq chunks
DT = DIM // 128  # 6  dim chunks / head pairs
VA = H * (DH + 1)  # 780: v with ones column per head
NPAIR = 6
N_DUMMY = 12

f32 = mybir.dt.float32
f32r = mybir.dt.float32r
bf16 = mybir.dt.bfloat16
EXP = mybir.ActivationFunctionType.Exp


def build():
    nc = bacc.Bacc("TRN2", target_bir_lowering=False, debug=False)
    xT_d = nc.dram_tensor("xT", [DIM, SEQ], f32r, kind="ExternalInput")
    wqkT_d = nc.dram_tensor("wqkT", [DIM, 2 * DIM], f32r, kind="ExternalInput")
    wvT_d = nc.dram_tensor("wvT", [DIM, DIM], f32r, kind="ExternalInput")
    wfcT_d = nc.dram_tensor("wfcT", [DIM, DIM], bf16, kind="ExternalInput")
    bfc_d = nc.dram_tensor("b_fc", [1, DIM], f32, kind="ExternalInput")
    out_d = nc.dram_tensor("out", [SEQ, DIM], f32, kind="ExternalOutput")

    with tile.TileContext(nc) as tc:
        with (
            tc.tile_pool(name="const", bufs=1) as constp,
            tc.tile_pool(name="persist", bufs=1) as persist,
            tc.tile_pool(name="work", bufs=1) as work,
            tc.tile_pool(name="dsc", bufs=1, space="DRAM") as dscp,
            tc.tile_pool(name="ps", bufs=1, space="PSUM") as psp,
        ):
            # ---- warm-up: dummy exp preloads ACT tables; full-array dummy
            # matmuls keep the PE busy (HAM at 8/8) while DMAs stream in.
            # The startup is DMA-bandwidth-bound for ~15us, so dummies are
            # also WOVEN between the pair-0 qk chains below (only after a
            # chain closes -- a dummy inside an open chain would deadlock
            # on the rotating "mm" PSUM buffers).
            dmy = constp.tile([128, 512], f32, tag="dmy")
            nc.gpsimd.memset(dmy[:], 0.25)
            dmy_o = constp.tile([1, 512], bf16, tag="dmyo")
            nc.scalar.activation(dmy_o[:], dmy[0:1, :], EXP)
            dmy_r = constp.tile([128, 512], f32r, tag="dmyr")
            nc.vector.tensor_copy(dmy_r[:], dmy[:])

            def dummy_run(n):
                for _ in range(n):
                    psd = psp.tile([128, 512], f32, tag="mm", bufs=2,
                                   name="warm")
                    nc.tensor.matmul(psd[:], dmy_r[:, 0:128], dmy_r[:],
                                     start=True, stop=True)

            dummy_run(N_DUMMY)

            wq = {}  # p -> [q_tile, k_tile]: each [128, 6*128] f32r, j-major

            def load_wq_half(p, ci):
                t = work.tile([128, DT * 128], f32r, tag=("wqq", "wqk")[ci],
                              bufs=2, name=f"wq{'qk'[ci]}{p}")
                nc.sync.dma_start(
                    t[:].rearrange("r (j c) -> r j c", c=128),
                    wqkT_d.ap()[:, ci * DIM + p * 128:ci * DIM + (p + 1) * 128]
                    .rearrange("(j r) c -> r j c", r=128))
                wq.setdefault(p, [None, None])[ci] = t

            def load_wq(p):
                load_wq_half(p, 0)
                load_wq_half(p, 1)

            # xT loaded in j-triple quarters so the first qk accumulation
            # (js 0-2, h2=0) can start as early as possible.
            xT_all = persist.tile([128, DT * SEQ], f32r, tag="xT", name="xT")
            xT3 = xT_all[:].rearrange("r (j c) -> r j c", c=SEQ)

            def xs(j, lo, hi):
                return xT_all[:, j * SEQ + lo:j * SEQ + hi]

            def load_x_part(h2, jlo, jhi):
                nc.sync.dma_start(
                    xT3[:, jlo:jhi, h2 * 512:(h2 + 1) * 512],
                    xT_d.ap()[jlo * 128:jhi * 128, h2 * 512:(h2 + 1) * 512]
                    .rearrange("(j r) c -> r j c", r=128))

            load_wq_half(0, 0)
            load_x_part(0, 0, 3)
            load_x_part(0, 3, 6)
            load_wq_half(0, 1)
            load_x_part(1, 0, 3)
            load_x_part(1, 3, 6)
            load_wq(1)

            bias_row = constp.tile([1, DIM], f32, tag="brow")
            nc.sync.dma_start(bias_row[:], bfc_d.ap())

            wvT_all = persist.tile([128, DT * DIM], f32r, tag="wvT",
                                   name="wvT")
            nc.sync.dma_start(
                wvT_all[:].rearrange("r (j c) -> r j c", c=DIM),
                wvT_d.ap().rearrange("(j r) c -> r j c", r=128))

            def vs(j, lo, hi):
                return wvT_all[:, j * DIM + lo:j * DIM + hi]

            wfcT_all = persist.tile([128, DT * DIM], bf16, tag="wfcT",
                                    name="wfcT")
            nc.sync.dma_start(
                wfcT_all[:].rearrange("r (j c) -> r j c", c=DIM),
                wfcT_d.ap().rearrange("(j r) c -> r j c", r=128))

            def ws(j, lo, hi):
                return wfcT_all[:, j * DIM + lo:j * DIM + hi]

            bias_bc = constp.tile([128, DIM], f32, tag="bbc")
            nc.gpsimd.partition_broadcast(bias_bc[:], bias_row[:],
                                          channels=128)

            # ---- persistent tensors
            va = [persist.tile([128, VA], bf16, tag=f"va{nt}", name=f"va{nt}")
                  for nt in range(NT)]
            aoT = [persist.tile([128, SEQ], bf16, tag=f"ao{j}", name=f"aoT{j}")
                   for j in range(DT)]
            fc_acc = [persist.tile([128, DIM], f32, tag=f"acc{nt}",
                                   name=f"acc{nt}") for nt in range(NT)]

            # ---- qkv matmul parts -------------------------------------
            def qk_parts(p, tiles):
                """q_T/k_T matmuls for pair p, split into 3-matmul halves
                so the background stream stays fine-grained."""
                def qkmm(half, ci, h2, js, box):
                    def go():
                        if js[0] == 0:
                            box["ps"] = psp.tile([128, 512], f32, tag="mm",
                                                 bufs=2, name="ps_qk")
                        ps = box["ps"]
                        src = wq[p][ci]
                        for j in js:
                            nc.tensor.matmul(
                                ps[:],
                                src[:, j * 128:(j + 1) * 128],
                                xs(j, h2 * 512, (h2 + 1) * 512),
                                start=(j == 0), stop=(j == DT - 1))
                        if js[-1] != DT - 1:
                            return
                        if h2 == 0:
                            t = work.tile([128, SEQ], f32r,
                                          tag=f"qk_{half}{p % 2}", bufs=1,
                                          name=f"qk{half}{p}")
                            tiles[half] = t
                        nc.vector.tensor_copy(
                            tiles[half][:, h2 * 512:(h2 + 1) * 512], ps[:])
                    return go

                def prefetch():
                    if p + 2 < NPAIR:
                        load_wq(p + 2)

                parts = []
                for half, ci, h2 in (("q", 0, 0), ("k", 1, 0),
                                     ("q", 0, 1), ("k", 1, 1)):
                    box = {}
                    parts.append(qkmm(half, ci, h2, (0, 1, 2), box))
                    parts.append(qkmm(half, ci, h2, (3, 4, 5), box))
                parts.append(prefetch)
                return parts

            def v_parts(nts):
                """v natural [128n, 12h x 64d] + ones column -> va (bf16)."""
                def vp(nt, h2):
                    lo, hi = (0, 512) if h2 == 0 else (512, 768)

                    def go():
                        psv = psp.tile([128, hi - lo], f32, tag="mm", bufs=2,
                                       name="psv")
                        for j in range(DT):
                            nc.tensor.matmul(psv[:],
                                             xs(j, nt * 128, (nt + 1) * 128),
                                             vs(j, lo, hi),
                                             start=(j == 0),
                                             stop=(j == DT - 1))
                        va3 = va[nt][:].rearrange("p (h c) -> p h c", c=DH + 1)
                        nc.vector.tensor_copy(
                            va3[:, lo // DH:hi // DH, 0:DH],
                            psv[:].rearrange("p (h c) -> p h c", c=DH))
                        if h2 == 1:
                            nc.gpsimd.memset(va3[:, :, DH:DH + 1], 1.0)
                    return go
                return [vp(nt, h2) for nt in nts for h2 in range(2)]

            # ---- fc accumulation passes: `js` head-pairs' contribution to
            # every seq-chunk accumulator.  One [128,768] PSUM tile per nt
            # (bank-aligned halves for the matmuls) and a SINGLE wide DVE
            # add -- the adds, not the matmuls, pace the fc tail.
            def fc_parts(js, first, last):
                def fp(nt):
                    def go():
                        psy = psp.tile([128, DIM], f32, tag="mm", bufs=2,
                                       name="psy")
                        for lo, hi in ((0, 512), (512, DIM)):
                            for i, j in enumerate(js):
                                nc.tensor.matmul(
                                    psy[:, lo:hi],
                                    aoT[j][:, nt * 128:(nt + 1) * 128],
                                    ws(j, lo, hi),
                                    start=(i == 0), stop=(i == len(js) - 1))
                        acc = fc_acc[nt]
                        if first:
                            nc.vector.tensor_add(acc[:], psy[:], bias_bc[:])
                        else:
                            nc.vector.tensor_add(acc[:], psy[:], acc[:])
                        if last:
                            nc.sync.dma_start(
                                out_d.ap()[nt * 128:(nt + 1) * 128, :],
                                acc[:])
                    return go
                return [fp(nt) for nt in range(NT)]

            # ---- softmax normalize: stage PSUM->SBUF, then a wide
            # reciprocal (a [1,1024] single-partition DVE op costs ~6.5us;
            # the [128,8] reshape keeps the reciprocal at ~0.2us).  The
            # reshape is ONE SBUF->SBUF partition-scatter DMA; the
            # partition-broadcast still needs a DRAM bounce.  For the tail
            # drains (pair 5) the staging copy runs on the Scalar engine,
            # which has finished all exps by then.
            def drain_po(p, xi, po, use_scalar=False):
                st = work.tile([DH + 1, SEQ], f32, tag="stage", bufs=2,
                               name="st")
                if use_scalar:
                    nc.scalar.copy(st[:], po[:])
                else:
                    nc.vector.tensor_copy(st[:], po[:])
                den8 = work.tile([128, 8], f32, tag="den8", bufs=2,
                                 name="den8")
                nc.sync.dma_start(den8[:], st[DH:DH + 1, :])
                recip8 = work.tile([128, 8], f32, tag="recip8", bufs=2,
                                   name="recip8")
                nc.vector.reciprocal(recip8[:], den8[:])
                dsc2 = dscp.tile([1, SEQ], f32, tag="dsc2", bufs=2,
                                 name="dsc2")
                nc.sync.dma_start(
                    dsc2[:].rearrange("a (p c) -> (a p) c", c=8), recip8[:])
                bc_sb = work.tile([64, SEQ], f32, tag="bc", bufs=2,
                                  name="bc_sb")
                nc.sync.dma_start(bc_sb[:], dsc2[:].broadcast_to([64, SEQ]))
                nc.vector.tensor_mul(
                    aoT[p][xi * 64:(xi + 1) * 64, :], st[0:DH, :], bc_sb[:])

            # ---- pipelined pair loop -----------------------------------
            def pair_step(p, qk, PT_prev, bg=()):
                """bg parts + PV(p-1) + S(p) quad + exp per chunk."""
                qt, kt = qk["q"], qk["k"]
                L = len(bg)
                PT = {}
                po = {}
                if PT_prev is not None:
                    for xi in range(2):
                        po[xi] = psp.tile([DH + 1, SEQ], f32, tag=f"o{xi}",
                                          bufs=1, name=f"po{xi}")
                def do_pv(c):
                    for xi in range(2):
                        hX = 2 * (p - 1) + xi
                        va_h = va[c][:, hX * (DH + 1):(hX + 1) * (DH + 1)]
                        for h2 in range(2):
                            nc.tensor.matmul(
                                po[xi][:, h2 * 512:(h2 + 1) * 512],
                                va_h,
                                PT_prev[(xi, c)][:, h2 * 512:
                                                 (h2 + 1) * 512],
                                start=(c == 0), stop=(c == NT - 1))

                def do_s(c):
                    ps = {}
                    for xi in range(2):
                        ps[xi] = psp.tile([128, SEQ], f32, tag="mm", bufs=2,
                                          name=f"ps_s{xi}")
                    for h2 in range(2):
                        for xi in range(2):
                            ro = xi * 64
                            nc.tensor.matmul(
                                ps[xi][:, h2 * 512:(h2 + 1) * 512],
                                kt[ro:ro + 64, c * 128:(c + 1) * 128],
                                qt[ro:ro + 64, h2 * 512:(h2 + 1) * 512],
                                start=True, stop=True,
                                tile_position=(ro, 0))
                    for xi in range(2):
                        pt = work.tile([128, SEQ], bf16, tag=f"pt{xi}_{c}",
                                       bufs=1, name="pt")
                        nc.scalar.activation(pt[:], ps[xi][:], EXP)
                        PT[(xi, c)] = pt

                for c in range(NT):
                    for i in range(L * c // NT, L * (c + 1) // NT):
                        bg[i]()
                    if PT_prev is not None:
                        do_pv(c)
                    do_s(c)
                if PT_prev is not None:
                    for xi in range(2):
                        drain_po(p - 1, xi, po[xi])
                return PT

            def pv_only(p, PT_prev, bg=()):
                """Last pair's PV, xi-major (no later exp depends on the pt
                tags).  The bg parts (fc pass 4) run in the xi=1 PV slots
                covering xi=0's drain chain; dummies after the xi=1 drain
                keep the HAM clock at 8/8 through the chain's ~10us flight
                so the final fc matmuls run at full rate."""
                L = len(bg)
                for xi in range(2):
                    po = psp.tile([DH + 1, SEQ], f32, tag="o0", bufs=1,
                                  name=f"po{xi}")
                    hX = 2 * p + xi
                    for t in range(4):
                        for c in (2 * t, 2 * t + 1):
                            va_h = va[c][:, hX * (DH + 1):
                                         (hX + 1) * (DH + 1)]
                            for h2 in range(2):
                                nc.tensor.matmul(
                                    po[:, h2 * 512:(h2 + 1) * 512],
                                    va_h,
                                    PT_prev[(xi, c)][:, h2 * 512:
                                                     (h2 + 1) * 512],
                                    start=(c == 0), stop=(c == NT - 1))
                        if xi == 1:
                            for i in range(L * t // 4, L * (t + 1) // 4):
                                bg[i]()
                    drain_po(p, xi, po, use_scalar=True)
                    if xi == 1:
                        dummy_run(20)
                for f in fc_parts((p,), first=False, last=True):
                    f()

            def merge(a, b):
                out, ia, ib = [], 0, 0
                while ia < len(a) or ib < len(b):
                    if ia * len(b) <= ib * len(a) and ia < len(a):
                        out.append(a[ia]); ia += 1
                    elif ib < len(b):
                        out.append(b[ib]); ib += 1
                    else:
                        out.append(a[ia]); ia += 1
                return out

            qk_tiles = {p: {} for p in range(NPAIR)}
            qk0 = qk_parts(0, qk_tiles[0])
            for i, f in enumerate(qk0):
                f()
                if i in (1, 3, 5):  # after each closed qk chain
                    dummy_run(10)

            # fc accumulation spread across the pair loop: pass {0,1} as
            # soon as aoT[1] exists, {2,3} in pair 5, {4} during PV(5),
            # {5} at the very end.  v-parts are concatenated AFTER qk parts
            # so early background work never waits on DMAs that arrive late
            # in the startup stream.
            bg_sched = {
                0: qk_parts(1, qk_tiles[1]) + v_parts(range(0, 6)),
                1: v_parts(range(6, 8)) + qk_parts(2, qk_tiles[2]),
                2: qk_parts(3, qk_tiles[3]),
                3: merge(qk_parts(4, qk_tiles[4]),
                         fc_parts((0, 1), first=True, last=False)),
                4: qk_parts(5, qk_tiles[5]),
                5: fc_parts((2, 3), first=False, last=False),
            }
            PT_cur = None
            for p in range(NPAIR):
                PT_cur = pair_step(p, qk_tiles[p], PT_cur, bg_sched[p])
            pv_only(5, PT_cur, bg=fc_parts((4,), first=False, last=False))

    nc.compile()
    return nc


_NC = None
LAST_RESULTS = None  # BassKernelResults of the most recent run (for profiling)


def kernel(**inputs) -> np.ndarray:
    global _NC, LAST_RESULTS
    x = np.asarray(inputs["x"], dtype=np.float32)
    w_qkv = np.asarray(inputs["w_qkv"], dtype=np.float32)
    w_fc = np.asarray(inputs["w_fc"], dtype=np.float32)
    b_fc = np.ascontiguousarray(
        np.asarray(inputs["b_fc"], dtype=np.float32).reshape(1, DIM))

    wqkT = np.ascontiguousarray(w_qkv[0:2 * DIM].T)    # [768, 1536]
    wvT = np.ascontiguousarray(w_qkv[2 * DIM:].T)      # [768, 768]
    wfcT = np.ascontiguousarray(w_fc.T).astype(ml_dtypes.bfloat16)

    if _NC is None:
        _NC = build()
    nc = _NC

    in_maps = [
        {"xT": np.ascontiguousarray(x[b].T), "wqkT": wqkT, "wvT": wvT,
         "wfcT": wfcT, "b_fc": b_fc}
        for b in range(8)
    ]
    res = run_bass_kernel_spmd(nc, in_maps, core_ids=list(range(8)))
    LAST_RESULTS = res
    out = np.stack([r["out"] for r in res.results], axis=0)
    return out.astype(np.float32)


if __name__ == "__main__":
    rng = np.random.default_rng(0)
    ins = {
        "x": rng.standard_normal((8, SEQ, DIM), dtype=np.float32),
        "w_qkv": (rng.standard_normal((3 * DIM, DIM), dtype=np.float32)
                  * DIM ** -0.5),
        "w_fc": (rng.standard_normal((DIM, DIM), dtype=np.float32)
                 * DIM ** -0.5),
        "b_fc": (rng.standard_normal((DIM,), dtype=np.float32) * 0.02),
    }
    out = kernel(**ins)
    print("out", out.shape, out.dtype)
